# revision 1
# baseline (speedup 1.0000x reference)
"""DeepSeekV2 MoE layer on 8 trn2 NeuronCores (expert-parallel).

Strategy (v3):
  - Host: gate softmax + group-limited top-k routing -> per-expert token index
    lists and combine weights (control data only; all heavy FLOPs on device).
  - Device (SPMD over 8 cores, 4 experts each; expert groups == cores):
      zero 4 column-sharded routed-partial tensors y_n [T, 512] (Scalar queue);
      per expert: transposed fp16 dma_gather per token chunk (tokens land
      H-tiled on partitions) -> mm1/mm3 fp16 -> silu*mul -> fp16 mm4 ->
      scale by combine weight -> one batched dma_scatter_add per (e, n);
      shared-expert intermediate (fp32r) for own 512-token slice;
      4x ReduceScatter(add) over cores (routed only) overlap the shared
      output matmuls; out = rs_n + shared.
  - Host: concatenate 512-row slices -> [B, S, H].
"""
import sys

import numpy as np

sys.path.insert(0, "/opt/trn_rl_repo")

import concourse.bass as bass
import concourse.mybir as mybir
import concourse.tile as tile
from concourse import bacc
from concourse.bass_utils import run_bass_kernel_spmd

F32 = mybir.dt.float32
F32R = mybir.dt.float32r
FP16 = mybir.dt.float16
I16 = mybir.dt.int16
AF = mybir.ActivationFunctionType
OP = mybir.AluOpType

N_GROUP, TOPK_GROUP, TOP_K = 8, 3, 6
NCORES = 8


def _routing(x, gate_w):
    T, E = x.shape[0], gate_w.shape[0]
    logits = (x @ gate_w.T).astype(np.float64)
    e = np.exp(logits - logits.max(-1, keepdims=True))
    scores = e / e.sum(-1, keepdims=True)
    per_group = E // N_GROUP
    group_scores = scores.reshape(T, N_GROUP, per_group).max(-1)
    order = np.argsort(-group_scores, axis=-1, kind="stable")
    group_mask = np.zeros((T, N_GROUP), bool)
    np.put_along_axis(group_mask, order[:, :TOPK_GROUP], True, axis=1)
    tmp = np.where(np.repeat(group_mask, per_group, axis=1), scores, 0.0)
    order_e = np.argsort(-tmp, axis=-1, kind="stable")
    topk_idx = order_e[:, :TOP_K]
    topk_w = np.take_along_axis(tmp, topk_idx, axis=1)
    topk_w = topk_w / (topk_w.sum(-1, keepdims=True) + 1e-20)
    combine = np.zeros((T, E), np.float32)
    np.put_along_axis(combine, topk_idx, topk_w.astype(np.float32), axis=1)
    return combine


def _chunks(cap):
    out, rem = [], cap
    while rem:
        if rem <= 512:
            out.append(rem)
            rem = 0
        elif rem == 640:
            out.append(384)
            rem = 256
        else:
            out.append(512)
            rem -= 512
    return out


def build_kernel(T, H, I, EPC, CAP, SI, act=AF.Silu, compile_=True):
    KT = H // 128         # H contraction tiles
    MT = I // 128         # I tiles
    CT = CAP // 128       # token tiles per expert
    N4 = max(H // 512, 1)
    NW = min(H, 512)
    SIT = SI // 128       # shared-intermediate tiles
    TOUT = T // NCORES    # own token slice
    TS = TOUT // 128
    CHUNKS = _chunks(CAP)
    MAXCW = max(CHUNKS)

    nc = bacc.Bacc("TRN2")
    x16 = nc.dram_tensor("x16", [T, H], FP16, kind="ExternalInput")
    xTc = nc.dram_tensor("xTc", [128, KT * TOUT], FP16, kind="ExternalInput")
    w13 = nc.dram_tensor("w13", [EPC, MT, 128, KT * 256], FP16, kind="ExternalInput")
    w2b = nc.dram_tensor("w2b", [EPC, N4, 128, MT * NW], FP16, kind="ExternalInput")
    sw13 = nc.dram_tensor("sw13", [SIT, 128, KT * 256], FP16, kind="ExternalInput")
    sw2b = nc.dram_tensor("sw2b", [N4, 128, SIT * NW], FP16, kind="ExternalInput")
    idx = nc.dram_tensor("idx", [EPC, 128, CAP // 16], I16, kind="ExternalInput")
    idxs = nc.dram_tensor("idxs", [EPC, 128, CAP // 16], I16, kind="ExternalInput")
    gat = nc.dram_tensor("gat", [EPC, 128, CT], F32, kind="ExternalInput")
    out = nc.dram_tensor("out", [TOUT, H], F32, kind="ExternalOutput")

    y_n = [nc.dram_tensor(f"y_col{n}", [T + 128, NW], FP16) for n in range(N4)]
    rs_n = [nc.dram_tensor(f"rs_col{n}", [TOUT, NW], FP16) for n in range(N4)]

    with tile.TileContext(nc) as tc:
        with (
            tc.tile_pool(name="const", bufs=1) as const,
            tc.tile_pool(name="persist", bufs=1) as persist,
            tc.tile_pool(name="xgtp", bufs=2) as xgtp,
            tc.tile_pool(name="xgtp1", bufs=1) as xgtp1,
            tc.tile_pool(name="gp", bufs=2) as gp,
            tc.tile_pool(name="stream", bufs=2) as stream,
            tc.tile_pool(name="one", bufs=1) as one,
            tc.tile_pool(name="small", bufs=2) as small,
            tc.tile_pool(name="psum", bufs=2, space="PSUM") as psum,
        ):
            idx_sb = const.tile([128, EPC, CAP // 16], I16)
            nc.sync.dma_start(idx_sb[:], idx.rearrange("e p c -> p e c"))
            idxs_sb = const.tile([128, EPC, CAP // 16], I16)
            nc.sync.dma_start(idxs_sb[:], idxs.rearrange("e p c -> p e c"))
            gat_sb = const.tile([128, EPC, CT], F32)
            nc.sync.dma_start(gat_sb[:], gat.rearrange("e p c -> p e c"))
            # shared-expert inputs, loaded up-front (Sync queue)
            xtc_sb = persist.tile([128, KT, TOUT], FP16)
            xtc_view = xTc.rearrange("p (k t) -> p k t", t=TOUT)
            for k in range(KT):
                nc.scalar.dma_start(xtc_sb[:, k:k + 1, :], xtc_view[:, k:k + 1, :])
            gs = persist.tile([128, SIT, TOUT], FP16)

            # ---------------- shared intermediate first (hides gather latency)
            for sm in range(SIT):
                s13 = stream.tile([128, KT, 256], FP16, tag="s13")
                nc.scalar.dma_start(
                    s13[:], sw13[sm].rearrange("p (k c) -> p k c", c=256))
                p1 = psum.tile([128, 512], F32, tag="p1")
                p3 = psum.tile([128, 512], F32, tag="p3")
                for k in range(KT):
                    nc.tensor.matmul(p1[:, :TOUT], s13[:, k, :128], xtc_sb[:, k, :],
                                     start=(k == 0), stop=(k == KT - 1))
                for k in range(KT):
                    nc.tensor.matmul(p3[:, :TOUT], s13[:, k, 128:], xtc_sb[:, k, :],
                                     start=(k == 0), stop=(k == KT - 1))
                nc.scalar.activation(gs[:, sm, :], p1[:, :TOUT], act)
                nc.vector.tensor_tensor(gs[:, sm, :], gs[:, sm, :], p3[:, :TOUT],
                                        OP.mult)
            ztile = const.tile([128, NW], FP16)
            nc.vector.memset(ztile[:], 0.0)
            for n in range(N4):
                for b in range(T // 128 + 1):
                    nc.scalar.dma_start(y_n[n][b * 128:(b + 1) * 128, :], ztile[:])
            # ---------------- routed experts --------------------------------
            for e in range(EPC):
                xgt_c = []
                c0 = 0
                for ci, cw in enumerate(CHUNKS):
                    pool_ci = xgtp if ci == 0 else xgtp1
                    xgt = pool_ci.tile([128, KT, cw], FP16, tag=f"xgt{ci}")
                    nc.gpsimd.dma_gather(
                        xgt[:], x16[:],
                        idx_sb[:, e, c0 // 16:(c0 + cw) // 16],
                        cw, cw, H, transpose=True)
                    xgt_c.append(xgt)
                    c0 += cw
                g = gp.tile([128, MT, CAP], FP16, tag="g")
                for m in range(MT):
                    w13t = stream.tile([128, KT, 256], FP16, tag="w13t")
                    nc.sync.dma_start(
                        w13t[:], w13[e, m].rearrange("p (k c) -> p k c", c=256))
                    c0 = 0
                    for ci, cw in enumerate(CHUNKS):
                        p1 = psum.tile([128, 512], F32, tag="p1")
                        p3 = psum.tile([128, 512], F32, tag="p3")
                        for k in range(KT):
                            nc.tensor.matmul(p1[:, :cw], w13t[:, k, :128],
                                             xgt_c[ci][:, k, :cw],
                                             start=(k == 0), stop=(k == KT - 1))
                        for k in range(KT):
                            nc.tensor.matmul(p3[:, :cw], w13t[:, k, 128:],
                                             xgt_c[ci][:, k, :cw],
                                             start=(k == 0), stop=(k == KT - 1))
                        nc.scalar.activation(g[:, m, c0:c0 + cw], p1[:, :cw], act)
                        nc.vector.tensor_tensor(g[:, m, c0:c0 + cw],
                                                g[:, m, c0:c0 + cw],
                                                p3[:, :cw], OP.mult)
                        c0 += cw
                for n in range(N4):
                    w2t = stream.tile([128, MT, NW], FP16, tag="w2t")
                    nc.sync.dma_start(
                        w2t[:], w2b[e, n].rearrange("p (k c) -> p k c", c=NW))
                    yb = stream.tile([128, CT, NW], FP16, tag="yb")
                    for ct in range(CT):
                        p4 = psum.tile([128, NW], F32, tag="p4")
                        for k2 in range(MT):
                            nc.tensor.matmul(p4[:], g[:, k2, ct * 128:(ct + 1) * 128],
                                             w2t[:, k2, :],
                                             start=(k2 == 0), stop=(k2 == MT - 1))
                        nc.vector.tensor_tensor(
                            yb[:, ct, :], p4[:],
                            gat_sb[:, e, ct:ct + 1].to_broadcast([128, NW]),
                            OP.mult)
                    nc.gpsimd.dma_scatter_add(
                        y_n[n][:], yb[:], idxs_sb[:, e, :], CAP, CAP, NW)

            # ---------------- shared intermediate (overlaps nothing yet) ----

            # ---------------- combine: 4x ReduceScatter (routed only) -------
            for n in range(N4):
                nc.gpsimd.collective_compute(
                    "ReduceScatter", OP.add,
                    replica_groups=[list(range(NCORES))],
                    ins=[y_n[n][0:T, :]],
                    outs=[rs_n[n][:]],
                )

            # ---------------- shared out + combine with rs ------------------
            for n in range(N4):
                s2 = stream.tile([128, SIT, NW], FP16, tag="s2")
                nc.scalar.dma_start(
                    s2[:], sw2b[n].rearrange("p (k c) -> p k c", c=NW))
                for ts in range(TS):
                    po = psum.tile([128, NW], F32, tag="p4")
                    for k2 in range(SIT):
                        nc.tensor.matmul(po[:], gs[:, k2, ts * 128:(ts + 1) * 128],
                                         s2[:, k2, :],
                                         start=(k2 == 0), stop=(k2 == SIT - 1))
                    rst = small.tile([128, NW], FP16, tag="rst")
                    nc.scalar.dma_start(rst[:], rs_n[n][ts * 128:(ts + 1) * 128, :])
                    ott = small.tile([128, NW], F32, tag="ott")
                    nc.vector.tensor_tensor(ott[:], po[:], rst[:], OP.add)
                    nc.sync.dma_start(
                        out[ts * 128:(ts + 1) * 128, n * NW:(n + 1) * NW], ott[:])

    if compile_:
        nc.compile()
    else:
        nc.insert_library_loads()
    return nc


def host_prep(hidden_states, gate_weight, w1, w2, w3, sw1, sw2, sw3):
    B, S, H = hidden_states.shape
    T = B * S
    E, I = w1.shape[0], w1.shape[1]
    SI = sw1.shape[0]
    EPC = E // NCORES
    KT, MT, SIT = H // 128, I // 128, SI // 128
    N4 = max(H // 512, 1)
    NW = min(H, 512)
    TOUT = T // NCORES

    x = np.ascontiguousarray(hidden_states.reshape(T, H), dtype=np.float32)
    combine = _routing(x, gate_weight.astype(np.float32))
    tok_lists = [np.nonzero(combine[:, e])[0] for e in range(E)]
    max_c = max(len(t) for t in tok_lists)
    CAP = max(128, ((max_c + 127) // 128) * 128)
    CT = CAP // 128

    x16 = x.astype(np.float16)
    xT = x.T  # [H, T] view

    s1 = sw1.T.reshape(KT, 128, SIT, 128).transpose(2, 1, 0, 3)
    s3 = sw3.T.reshape(KT, 128, SIT, 128).transpose(2, 1, 0, 3)
    sw13 = np.ascontiguousarray(
        np.concatenate([s1, s3], axis=-1).reshape(SIT, 128, -1), dtype=np.float16)
    sw2b = np.ascontiguousarray(
        sw2.T.reshape(SIT, 128, N4, NW).transpose(2, 1, 0, 3).reshape(N4, 128, -1),
        dtype=np.float16)

    in_maps = []
    for c in range(NCORES):
        els = list(range(c * EPC, (c + 1) * EPC))
        idx_np = np.zeros((EPC, 128, CAP // 16), np.int16)
        idxs_np = np.zeros((EPC, 128, CAP // 16), np.int16)
        gat_np = np.zeros((EPC, 128, CT), np.float32)
        for j, e in enumerate(els):
            toks = tok_lists[e]
            a = np.zeros(CAP, np.int16)
            a[:len(toks)] = toks
            idx_np[j] = np.tile(a.reshape(CAP // 16, 16).T, (8, 1))
            b2 = np.full(CAP, T, np.int16)
            b2[:len(toks)] = toks
            idxs_np[j] = np.tile(b2.reshape(CAP // 16, 16).T, (8, 1))
            gv = np.zeros(CAP, np.float32)
            gv[:len(toks)] = combine[toks, e]
            gat_np[j] = gv.reshape(CT, 128).T
        w13c = np.empty((EPC, MT, 128, KT * 256), np.float16)
        w2c = np.empty((EPC, N4, 128, MT * NW), np.float16)
        for j, e in enumerate(els):
            a1 = w1[e].T.reshape(KT, 128, MT, 128).transpose(2, 1, 0, 3)
            a3 = w3[e].T.reshape(KT, 128, MT, 128).transpose(2, 1, 0, 3)
            w13c[j] = np.concatenate([a1, a3], axis=-1).reshape(MT, 128, -1)
            w2c[j] = (w2[e].T.reshape(MT, 128, N4, NW)
                      .transpose(2, 1, 0, 3).reshape(N4, 128, -1))
        xTc = np.ascontiguousarray(
            xT[:, c * TOUT:(c + 1) * TOUT].reshape(KT, 128, TOUT)
            .transpose(1, 0, 2).reshape(128, -1), dtype=np.float16)
        in_maps.append({
            "x16": x16, "xTc": xTc,
            "w13": w13c, "w2b": w2c,
            "sw13": sw13, "sw2b": sw2b,
            "idx": idx_np, "idxs": idxs_np, "gat": gat_np,
        })
    cfg = dict(T=T, H=H, I=I, EPC=EPC, CAP=CAP, SI=SI)
    return in_maps, cfg


def kernel(**inputs):
    inputs = {k: np.asarray(v) for k, v in inputs.items()}
    hs = inputs["hidden_states"]
    B, S, H = hs.shape
    in_maps, cfg = host_prep(
        hs, inputs["gate_weight"], inputs["w1"], inputs["w2"], inputs["w3"],
        inputs["sw1"], inputs["sw2"], inputs["sw3"])
    nc = build_kernel(**cfg)
    res = run_bass_kernel_spmd(nc, in_maps, list(range(NCORES)))
    y = np.concatenate([res.results[c]["out"] for c in range(NCORES)], axis=0)
    return y.reshape(B, S, H).astype(np.float32)


if __name__ == "__main__":
    pass



# revision 4
# speedup vs baseline: 1.0048x; 1.0048x over previous
"""DeepSeekV2 MoE layer on 8 trn2 NeuronCores (expert-parallel).

Strategy (v4):
  - Host: gate softmax + group-limited top-k routing -> per-expert token index
    lists and combine weights (control data only; all heavy FLOPs on device).
    Experts are load-balanced across cores (serpentine over counts) and each
    core's 4 expert slots get per-slot capacities (max over cores, ceil 128).
  - Device (SPMD over 8 cores, 4 expert slots each):
      A: per slot: transposed fp16 dma_gather (double-buffered) -> mm1/mm3
         fp16 -> silu*mul -> g[slot] kept in SBUF;
      B: column-major down-proj: for each 512-wide output column: all 4 slots'
         mm2 + gate-scale + dma_scatter_add into y_col, then ReduceScatter(add)
         for that column -> the 4 RS's overlap the shared-expert phase;
      S: shared-expert intermediate for own 512-token slice (runs after B so
         the RS chain hides under it; first 2 iters run up-front to cover
         gather latency);
      C: shared out matmuls + add RS result -> out.
  - Host: concatenate 512-row slices -> [B, S, H].
"""
import sys

import numpy as np

sys.path.insert(0, "/opt/trn_rl_repo")

import concourse.bass as bass
import concourse.mybir as mybir
import concourse.tile as tile
from concourse import bacc
from concourse.bass_utils import run_bass_kernel_spmd

F32 = mybir.dt.float32
FP16 = mybir.dt.float16
I16 = mybir.dt.int16
AF = mybir.ActivationFunctionType
OP = mybir.AluOpType

N_GROUP, TOPK_GROUP, TOP_K = 8, 3, 6
NCORES = 8
S1A = 2  # shared-intermediate iters run before phase A (warmup filler)


def _routing(x, gate_w):
    T, E = x.shape[0], gate_w.shape[0]
    logits = (x @ gate_w.T).astype(np.float64)
    e = np.exp(logits - logits.max(-1, keepdims=True))
    scores = e / e.sum(-1, keepdims=True)
    per_group = E // N_GROUP
    group_scores = scores.reshape(T, N_GROUP, per_group).max(-1)
    order = np.argsort(-group_scores, axis=-1, kind="stable")
    group_mask = np.zeros((T, N_GROUP), bool)
    np.put_along_axis(group_mask, order[:, :TOPK_GROUP], True, axis=1)
    tmp = np.where(np.repeat(group_mask, per_group, axis=1), scores, 0.0)
    order_e = np.argsort(-tmp, axis=-1, kind="stable")
    topk_idx = order_e[:, :TOP_K]
    topk_w = np.take_along_axis(tmp, topk_idx, axis=1)
    topk_w = topk_w / (topk_w.sum(-1, keepdims=True) + 1e-20)
    combine = np.zeros((T, E), np.float32)
    np.put_along_axis(combine, topk_idx, topk_w.astype(np.float32), axis=1)
    return combine


def _chunks(cap):
    out, rem = [], cap
    while rem:
        if rem <= 512:
            out.append(rem)
            rem = 0
        elif rem == 640:
            out.append(384)
            rem = 256
        else:
            out.append(512)
            rem -= 512
    return out


def build_kernel(T, H, I, CAPS, SI, act=AF.Silu, compile_=True):
    EPC = len(CAPS)
    KT = H // 128         # H contraction tiles
    MT = I // 128         # I tiles
    N4 = max(H // 512, 1)
    NW = min(H, 512)
    SIT = SI // 128       # shared-intermediate tiles
    TOUT = T // NCORES    # own token slice
    TS = TOUT // 128
    CAP0 = max(CAPS)
    CT0 = CAP0 // 128
    CHUNKS = [_chunks(c) for c in CAPS]

    nc = bacc.Bacc("TRN2")
    x16 = nc.dram_tensor("x16", [T, H], FP16, kind="ExternalInput")
    xTc = nc.dram_tensor("xTc", [128, KT * TOUT], FP16, kind="ExternalInput")
    w13 = nc.dram_tensor("w13", [EPC, MT, 128, KT * 256], FP16, kind="ExternalInput")
    w2b = nc.dram_tensor("w2b", [EPC, N4, 128, MT * NW], FP16, kind="ExternalInput")
    sw13 = nc.dram_tensor("sw13", [SIT, 128, KT * 256], FP16, kind="ExternalInput")
    sw2b = nc.dram_tensor("sw2b", [N4, 128, SIT * NW], FP16, kind="ExternalInput")
    idx = nc.dram_tensor("idx", [EPC, 128, CAP0 // 16], I16, kind="ExternalInput")
    idxs = nc.dram_tensor("idxs", [EPC, 128, CAP0 // 16], I16, kind="ExternalInput")
    gat = nc.dram_tensor("gat", [EPC, 128, CT0], F32, kind="ExternalInput")
    out = nc.dram_tensor("out", [TOUT, H], F32, kind="ExternalOutput")

    y_n = [nc.dram_tensor(f"y_col{n}", [T + 128, NW], FP16) for n in range(N4)]
    rs_n = [nc.dram_tensor(f"rs_col{n}", [TOUT, NW], FP16) for n in range(N4)]

    with tile.TileContext(nc) as tc:
        with (
            tc.tile_pool(name="const", bufs=1) as const,
            tc.tile_pool(name="persist", bufs=1) as persist,
            tc.tile_pool(name="xgtp", bufs=2) as xgtp,
            tc.tile_pool(name="wstream", bufs=2) as wstream,
            tc.tile_pool(name="s2p", bufs=1) as s2p,
            tc.tile_pool(name="ybp", bufs=2) as ybp,
            tc.tile_pool(name="small", bufs=2) as small,
            tc.tile_pool(name="psum", bufs=2, space="PSUM") as psum,
        ):
            idx_sb = const.tile([128, EPC, CAP0 // 16], I16)
            nc.scalar.dma_start(idx_sb[:], idx.rearrange("e p c -> p e c"))
            idxs_sb = const.tile([128, EPC, CAP0 // 16], I16)
            nc.scalar.dma_start(idxs_sb[:], idxs.rearrange("e p c -> p e c"))
            gat_sb = const.tile([128, EPC, CT0], F32)
            nc.scalar.dma_start(gat_sb[:], gat.rearrange("e p c -> p e c"))
            # shared-expert input (own tokens, H-tiled on partitions)
            xtc_sb = persist.tile([128, KT, TOUT], FP16)
            xtc_view = xTc.rearrange("p (k t) -> p k t", t=TOUT)
            for k in range(KT):
                nc.scalar.dma_start(xtc_sb[:, k:k + 1, :], xtc_view[:, k:k + 1, :])
            gs = persist.tile([128, SIT, TOUT], FP16)
            g_sl = [persist.tile([128, MT, CAPS[j]], FP16, tag=f"g{j}",
                                 name=f"g{j}")
                    for j in range(EPC)]

            def shared_int(sm):
                s13 = wstream.tile([128, KT, 256], FP16, tag="w")
                nc.sync.dma_start(
                    s13[:], sw13[sm].rearrange("p (k c) -> p k c", c=256))
                p1 = psum.tile([128, 512], F32, tag="p1")
                p3 = psum.tile([128, 512], F32, tag="p3")
                for k in range(KT):
                    nc.tensor.matmul(p1[:, :TOUT], s13[:, k, :128], xtc_sb[:, k, :],
                                     start=(k == 0), stop=(k == KT - 1))
                for k in range(KT):
                    nc.tensor.matmul(p3[:, :TOUT], s13[:, k, 128:], xtc_sb[:, k, :],
                                     start=(k == 0), stop=(k == KT - 1))
                nc.scalar.activation(gs[:, sm, :], p1[:, :TOUT], act)
                nc.vector.tensor_tensor(gs[:, sm, :], gs[:, sm, :], p3[:, :TOUT],
                                        OP.mult)

            # zero routed accumulators (scalar queue; must finish before phase B)
            ztile = const.tile([128, NW], FP16)
            nc.vector.memset(ztile[:], 0.0)
            for n in range(N4):
                for b in range(T // 128 + 1):
                    nc.scalar.dma_start(y_n[n][b * 128:(b + 1) * 128, :], ztile[:])

            # a couple of shared-intermediate iters to cover gather latency
            for sm in range(S1A):
                shared_int(sm)

            # ---------------- phase A: gathers + up-proj -> g ---------------
            for j in range(EPC):
                capj = CAPS[j]
                xgt_c = []
                c0 = 0
                for ci, cw in enumerate(CHUNKS[j]):
                    xgt = xgtp.tile([128, KT, cw], FP16, tag=f"xg{ci}")
                    nc.gpsimd.dma_gather(
                        xgt[:], x16[:],
                        idx_sb[:, j, c0 // 16:(c0 + cw) // 16],
                        cw, cw, H, transpose=True)
                    xgt_c.append(xgt)
                    c0 += cw
                for m in range(MT):
                    w13t = wstream.tile([128, KT, 256], FP16, tag="w")
                    nc.sync.dma_start(
                        w13t[:], w13[j, m].rearrange("p (k c) -> p k c", c=256))
                    c0 = 0
                    for ci, cw in enumerate(CHUNKS[j]):
                        p1 = psum.tile([128, 512], F32, tag="p1")
                        p3 = psum.tile([128, 512], F32, tag="p3")
                        for k in range(KT):
                            nc.tensor.matmul(p1[:, :cw], w13t[:, k, :128],
                                             xgt_c[ci][:, k, :cw],
                                             start=(k == 0), stop=(k == KT - 1))
                        for k in range(KT):
                            nc.tensor.matmul(p3[:, :cw], w13t[:, k, 128:],
                                             xgt_c[ci][:, k, :cw],
                                             start=(k == 0), stop=(k == KT - 1))
                        nc.scalar.activation(g_sl[j][:, m, c0:c0 + cw],
                                             p1[:, :cw], act)
                        nc.vector.tensor_tensor(g_sl[j][:, m, c0:c0 + cw],
                                                g_sl[j][:, m, c0:c0 + cw],
                                                p3[:, :cw], OP.mult)
                        c0 += cw

            # ---------------- phase B: column-major down-proj + per-col RS --
            for n in range(N4):
                for j in range(EPC):
                    capj = CAPS[j]
                    ctj = capj // 128
                    w2t = wstream.tile([128, MT, NW], FP16, tag="w")
                    nc.sync.dma_start(
                        w2t[:], w2b[j, n].rearrange("p (k c) -> p k c", c=NW))
                    yb = ybp.tile([128, ctj, NW], FP16, tag="yb")
                    for ct in range(ctj):
                        p4 = psum.tile([128, NW], F32, tag="p4")
                        for k2 in range(MT):
                            nc.tensor.matmul(p4[:],
                                             g_sl[j][:, k2, ct * 128:(ct + 1) * 128],
                                             w2t[:, k2, :],
                                             start=(k2 == 0), stop=(k2 == MT - 1))
                        nc.vector.tensor_tensor(
                            yb[:, ct, :], p4[:],
                            gat_sb[:, j, ct:ct + 1].to_broadcast([128, NW]),
                            OP.mult)
                    nc.gpsimd.dma_scatter_add(
                        y_n[n][:], yb[:], idxs_sb[:, j, :capj // 16],
                        capj, capj, NW)
                nc.gpsimd.collective_compute(
                    "ReduceScatter", OP.add,
                    replica_groups=[list(range(NCORES))],
                    ins=[y_n[n][0:T, :]],
                    outs=[rs_n[n][:]],
                )

            # ---------------- phase S: rest of shared intermediate ----------
            for sm in range(S1A, SIT):
                shared_int(sm)

            # ---------------- phase C: shared out + combine with rs ---------
            for n in range(N4):
                s2 = s2p.tile([128, SIT, NW], FP16, tag="s2")
                nc.scalar.dma_start(
                    s2[:], sw2b[n].rearrange("p (k c) -> p k c", c=NW))
                for ts in range(TS):
                    po = psum.tile([128, NW], F32, tag="p4")
                    for k2 in range(SIT):
                        nc.tensor.matmul(po[:], gs[:, k2, ts * 128:(ts + 1) * 128],
                                         s2[:, k2, :],
                                         start=(k2 == 0), stop=(k2 == SIT - 1))
                    rst = small.tile([128, NW], FP16, tag="rst")
                    nc.sync.dma_start(rst[:], rs_n[n][ts * 128:(ts + 1) * 128, :])
                    ott = small.tile([128, NW], F32, tag="ott")
                    nc.vector.tensor_tensor(ott[:], po[:], rst[:], OP.add)
                    nc.sync.dma_start(
                        out[ts * 128:(ts + 1) * 128, n * NW:(n + 1) * NW], ott[:])

    if compile_:
        nc.compile()
    else:
        nc.insert_library_loads()
    return nc


def host_prep(hidden_states, gate_weight, w1, w2, w3, sw1, sw2, sw3):
    B, S, H = hidden_states.shape
    T = B * S
    E, I = w1.shape[0], w1.shape[1]
    SI = sw1.shape[0]
    EPC = E // NCORES
    KT, MT, SIT = H // 128, I // 128, SI // 128
    N4 = max(H // 512, 1)
    NW = min(H, 512)
    TOUT = T // NCORES

    x = np.ascontiguousarray(hidden_states.reshape(T, H), dtype=np.float32)
    combine = _routing(x, gate_weight.astype(np.float32))
    tok_lists = [np.nonzero(combine[:, e])[0] for e in range(E)]
    counts = np.array([len(t) for t in tok_lists])

    # balance experts over cores: serpentine over count-sorted experts; slot s
    # on every core holds its s-th largest expert, so per-slot capacity is the
    # max over cores of that rank band.
    order = np.argsort(-counts, kind="stable")
    assign = np.zeros((NCORES, EPC), np.int64)
    for s in range(EPC):
        band = order[s * NCORES:(s + 1) * NCORES]
        if s % 2 == 1:
            band = band[::-1]
        for c in range(NCORES):
            assign[c, s] = band[c]
    CAPS = tuple(
        int(max(128, ((counts[assign[:, s]].max() + 127) // 128) * 128))
        for s in range(EPC))
    CAP0 = max(CAPS)
    CT0 = CAP0 // 128

    x16 = x.astype(np.float16)
    xT = x.T  # [H, T] view

    s1 = sw1.T.reshape(KT, 128, SIT, 128).transpose(2, 1, 0, 3)
    s3 = sw3.T.reshape(KT, 128, SIT, 128).transpose(2, 1, 0, 3)
    sw13 = np.ascontiguousarray(
        np.concatenate([s1, s3], axis=-1).reshape(SIT, 128, -1), dtype=np.float16)
    sw2b = np.ascontiguousarray(
        sw2.T.reshape(SIT, 128, N4, NW).transpose(2, 1, 0, 3).reshape(N4, 128, -1),
        dtype=np.float16)

    in_maps = []
    for c in range(NCORES):
        els = list(assign[c])
        idx_np = np.zeros((EPC, 128, CAP0 // 16), np.int16)
        idxs_np = np.zeros((EPC, 128, CAP0 // 16), np.int16)
        gat_np = np.zeros((EPC, 128, CT0), np.float32)
        for j, e in enumerate(els):
            cap = CAPS[j]
            toks = tok_lists[e]
            a = np.zeros(cap, np.int16)
            a[:len(toks)] = toks
            idx_np[j, :, :cap // 16] = np.tile(a.reshape(cap // 16, 16).T, (8, 1))
            b2 = np.full(cap, T, np.int16)
            b2[:len(toks)] = toks
            idxs_np[j, :, :cap // 16] = np.tile(b2.reshape(cap // 16, 16).T, (8, 1))
            gv = np.zeros(cap, np.float32)
            gv[:len(toks)] = combine[toks, e]
            gat_np[j, :, :cap // 128] = gv.reshape(cap // 128, 128).T
        w13c = np.empty((EPC, MT, 128, KT * 256), np.float16)
        w2c = np.empty((EPC, N4, 128, MT * NW), np.float16)
        for j, e in enumerate(els):
            a1 = w1[e].T.reshape(KT, 128, MT, 128).transpose(2, 1, 0, 3)
            a3 = w3[e].T.reshape(KT, 128, MT, 128).transpose(2, 1, 0, 3)
            w13c[j] = np.concatenate([a1, a3], axis=-1).reshape(MT, 128, -1)
            w2c[j] = (w2[e].T.reshape(MT, 128, N4, NW)
                      .transpose(2, 1, 0, 3).reshape(N4, 128, -1))
        xTc = np.ascontiguousarray(
            xT[:, c * TOUT:(c + 1) * TOUT].reshape(KT, 128, TOUT)
            .transpose(1, 0, 2).reshape(128, -1), dtype=np.float16)
        in_maps.append({
            "x16": x16, "xTc": xTc,
            "w13": w13c, "w2b": w2c,
            "sw13": sw13, "sw2b": sw2b,
            "idx": idx_np, "idxs": idxs_np, "gat": gat_np,
        })
    cfg = dict(T=T, H=H, I=I, CAPS=CAPS, SI=SI)
    return in_maps, cfg


def kernel(**inputs):
    inputs = {k: np.asarray(v) for k, v in inputs.items()}
    hs = inputs["hidden_states"]
    B, S, H = hs.shape
    in_maps, cfg = host_prep(
        hs, inputs["gate_weight"], inputs["w1"], inputs["w2"], inputs["w3"],
        inputs["sw1"], inputs["sw2"], inputs["sw3"])
    nc = build_kernel(**cfg)
    res = run_bass_kernel_spmd(nc, in_maps, list(range(NCORES)))
    y = np.concatenate([res.results[c]["out"] for c in range(NCORES)], axis=0)
    return y.reshape(B, S, H).astype(np.float32)


if __name__ == "__main__":
    pass


# revision 7
# speedup vs baseline: 1.1682x; 1.1626x over previous
"""DeepSeekV2 MoE layer on 8 trn2 NeuronCores (expert-parallel).

Strategy (v5):
  - Host: gate softmax + group-limited top-k routing -> per-expert token index
    lists and combine weights (control data only; all heavy FLOPs on device).
    Experts are load-balanced across cores (serpentine over counts) and each
    core's 4 expert slots get per-slot capacities (max over cores, ceil 128).
  - Device (SPMD over 8 cores, 4 expert slots each):
      A: per slot: transposed fp16 dma_gather (double-buffered, spread over
         4 SWDGE queues) -> mm1/mm3 fp16 -> silu*mul -> g[slot] in SBUF;
         zero-fill of the y accumulators is interleaved here (sync/scalar);
      B: column-PAIR major down-proj: for each 1024-wide column pair: all 4
         slots' mm2 + gate-scale + one 2KB-elem dma_scatter_add per slot
         (negative-index padding skips pad tokens), then ReduceScatter(add)
         for the pair -> the 2 RS's overlap the shared-expert phase;
      S: shared-expert intermediate for own 512-token slice (after B so the
         RS chain hides under it; first S1A iters run up-front as warmup
         filler while the first gather lands);
      C: shared out matmuls + add RS result -> out.
  - Host: concatenate 512-row slices -> [B, S, H].
"""
import sys

import numpy as np

sys.path.insert(0, "/opt/trn_rl_repo")

import concourse.bass as bass
import concourse.mybir as mybir
import concourse.tile as tile
from concourse import bacc
from concourse.bass_utils import run_bass_kernel_spmd

F32 = mybir.dt.float32
FP16 = mybir.dt.float16
I16 = mybir.dt.int16
AF = mybir.ActivationFunctionType
OP = mybir.AluOpType

N_GROUP, TOPK_GROUP, TOP_K = 8, 3, 6
NCORES = 8
S1A = 2  # shared-intermediate iters run before phase A (warmup filler)


def _routing(x, gate_w):
    T, E = x.shape[0], gate_w.shape[0]
    logits = (x @ gate_w.T).astype(np.float64)
    e = np.exp(logits - logits.max(-1, keepdims=True))
    scores = e / e.sum(-1, keepdims=True)
    per_group = E // N_GROUP
    group_scores = scores.reshape(T, N_GROUP, per_group).max(-1)
    order = np.argsort(-group_scores, axis=-1, kind="stable")
    group_mask = np.zeros((T, N_GROUP), bool)
    np.put_along_axis(group_mask, order[:, :TOPK_GROUP], True, axis=1)
    tmp = np.where(np.repeat(group_mask, per_group, axis=1), scores, 0.0)
    order_e = np.argsort(-tmp, axis=-1, kind="stable")
    topk_idx = order_e[:, :TOP_K]
    topk_w = np.take_along_axis(tmp, topk_idx, axis=1)
    topk_w = topk_w / (topk_w.sum(-1, keepdims=True) + 1e-20)
    combine = np.zeros((T, E), np.float32)
    np.put_along_axis(combine, topk_idx, topk_w.astype(np.float32), axis=1)
    return combine


def _chunks(cap):
    out, rem = [], cap
    while rem:
        if rem <= 512:
            out.append(rem)
            rem = 0
        elif rem == 640:
            out.append(384)
            rem = 256
        else:
            out.append(512)
            rem -= 512
    return out


def build_kernel(T, H, I, CAPS, SI, act=AF.Silu, compile_=True):
    EPC = len(CAPS)
    KT = H // 128         # H contraction tiles
    MT = I // 128         # I tiles
    NP = max(H // 1024, 1)  # column pairs
    PW = min(H, 1024)       # pair width
    NW = 512
    SIT = SI // 128       # shared-intermediate tiles
    TOUT = T // NCORES    # own token slice
    TS = TOUT // 128
    CAP0 = max(CAPS)
    CT0 = CAP0 // 128
    CHUNKS = [_chunks(c) for c in CAPS]
    ZBLK = (T + 128) // 128  # zero blocks per pair tensor

    nc = bacc.Bacc("TRN2")
    x16 = nc.dram_tensor("x16", [T, H], FP16, kind="ExternalInput")
    xTc = nc.dram_tensor("xTc", [128, KT * TOUT], FP16, kind="ExternalInput")
    w13 = nc.dram_tensor("w13", [EPC, MT, 128, KT * 256], FP16, kind="ExternalInput")
    w2b = nc.dram_tensor("w2b", [EPC, H // NW, 128, MT * NW], FP16,
                         kind="ExternalInput")
    sw13 = nc.dram_tensor("sw13", [SIT, 128, KT * 256], FP16, kind="ExternalInput")
    sw2b = nc.dram_tensor("sw2b", [H // NW, 128, SIT * NW], FP16,
                          kind="ExternalInput")
    idx = nc.dram_tensor("idx", [EPC, 128, CAP0 // 16], I16, kind="ExternalInput")
    idxs = nc.dram_tensor("idxs", [EPC, 128, CAP0 // 16], I16, kind="ExternalInput")
    gat = nc.dram_tensor("gat", [EPC, 128, CT0], F32, kind="ExternalInput")
    out = nc.dram_tensor("out", [TOUT, H], F32, kind="ExternalOutput")

    y_p = [nc.dram_tensor(f"y_pair{p}", [T + 128, PW], FP16) for p in range(NP)]
    rs_p = [nc.dram_tensor(f"rs_pair{p}", [TOUT, PW], FP16) for p in range(NP)]

    with tile.TileContext(nc) as tc:
        with (
            tc.tile_pool(name="const", bufs=1) as const,
            tc.tile_pool(name="persist", bufs=1) as persist,
            tc.tile_pool(name="xgtp", bufs=2) as xgtp,
            tc.tile_pool(name="wstream", bufs=3) as wstream,
            tc.tile_pool(name="ybp", bufs=2) as ybp,
            tc.tile_pool(name="small", bufs=2) as small,
            tc.tile_pool(name="psum", bufs=2, space="PSUM") as psum,
        ):
            idx_sb = const.tile([128, EPC, CAP0 // 16], I16)
            nc.scalar.dma_start(idx_sb[:], idx.rearrange("e p c -> p e c"))
            idxs_sb = const.tile([128, EPC, CAP0 // 16], I16)
            nc.scalar.dma_start(idxs_sb[:], idxs.rearrange("e p c -> p e c"))
            gat_sb = const.tile([128, EPC, CT0], F32)
            nc.scalar.dma_start(gat_sb[:], gat.rearrange("e p c -> p e c"))
            # shared-expert input (own tokens, H-tiled on partitions)
            xtc_sb = persist.tile([128, KT, TOUT], FP16)
            nc.scalar.dma_start(xtc_sb[:], xTc[:])
            gs = persist.tile([128, SIT, TOUT], FP16)
            g_sl = [persist.tile([128, MT, CAPS[j]], FP16, tag=f"g{j}",
                                 name=f"g{j}")
                    for j in range(EPC)]
            ztile = const.tile([128, PW], FP16)
            nc.vector.memset(ztile[:], 0.0)

            def shared_int(sm):
                s13 = wstream.tile([128, KT, 256], FP16, tag="w")
                nc.sync.dma_start(
                    s13[:], sw13[sm].rearrange("p (k c) -> p k c", c=256))
                p1 = psum.tile([128, 512], F32, tag="p1")
                p3 = psum.tile([128, 512], F32, tag="p3")
                for k in range(KT):
                    nc.tensor.matmul(p1[:, :TOUT], s13[:, k, :128], xtc_sb[:, k, :],
                                     start=(k == 0), stop=(k == KT - 1))
                for k in range(KT):
                    nc.tensor.matmul(p3[:, :TOUT], s13[:, k, 128:], xtc_sb[:, k, :],
                                     start=(k == 0), stop=(k == KT - 1))
                nc.scalar.activation(gs[:, sm, :], p1[:, :TOUT], act)
                nc.vector.tensor_tensor(gs[:, sm, :], gs[:, sm, :], p3[:, :TOUT],
                                        OP.mult)

            # warmup filler while the first gathers land
            for sm in range(S1A):
                shared_int(sm)

            # ---------------- phase A: gathers + up-proj -> g ---------------
            # (zero-fill of y pairs is interleaved per expert: sync/scalar)
            for j in range(EPC):
                xgt_c = []
                c0 = 0
                for ci, cw in enumerate(CHUNKS[j]):
                    xgt = xgtp.tile([128, KT, cw], FP16, tag=f"xg{ci}",
                                    name=f"xg{ci}")
                    nc.gpsimd.dma_gather(
                        xgt[:], x16[:],
                        idx_sb[:, j, c0 // 16:(c0 + cw) // 16],
                        cw, cw, H, transpose=True)
                    xgt_c.append(xgt)
                    c0 += cw
                for m in range(MT):
                    w13t = wstream.tile([128, KT, 256], FP16, tag="w")
                    nc.sync.dma_start(
                        w13t[:], w13[j, m].rearrange("p (k c) -> p k c", c=256))
                    c0 = 0
                    for ci, cw in enumerate(CHUNKS[j]):
                        p1 = psum.tile([128, 512], F32, tag="p1")
                        p3 = psum.tile([128, 512], F32, tag="p3")
                        for k in range(KT):
                            nc.tensor.matmul(p1[:, :cw], w13t[:, k, :128],
                                             xgt_c[ci][:, k, :cw],
                                             start=(k == 0), stop=(k == KT - 1))
                        for k in range(KT):
                            nc.tensor.matmul(p3[:, :cw], w13t[:, k, 128:],
                                             xgt_c[ci][:, k, :cw],
                                             start=(k == 0), stop=(k == KT - 1))
                        nc.scalar.activation(g_sl[j][:, m, c0:c0 + cw],
                                             p1[:, :cw], act)
                        nc.vector.tensor_tensor(g_sl[j][:, m, c0:c0 + cw],
                                                g_sl[j][:, m, c0:c0 + cw],
                                                p3[:, :cw], OP.mult)
                        c0 += cw
                # zero-fill: expert j covers half the blocks of pair j//2
                pz = j // (EPC // NP) if NP > 1 else 0
                half = j % (EPC // NP)
                blocks = range(half * ((ZBLK + 1) // 2),
                               min(ZBLK, (half + 1) * ((ZBLK + 1) // 2)))
                for bi, b in enumerate(blocks):
                    eng = nc.sync if bi % 2 == 0 else nc.scalar
                    eng.dma_start(y_p[pz][b * 128:(b + 1) * 128, :], ztile[:])

            # ---------------- phase B: pair-major down-proj + per-pair RS ---
            for p in range(NP):
                for j in range(EPC):
                    capj = CAPS[j]
                    ctj = capj // 128
                    w2a = wstream.tile([128, MT, NW], FP16, tag="w")
                    nc.scalar.dma_start(
                        w2a[:], w2b[j, 2 * p].rearrange("p (k c) -> p k c", c=NW))
                    w2c = wstream.tile([128, MT, NW], FP16, tag="w")
                    nc.scalar.dma_start(
                        w2c[:], w2b[j, 2 * p + 1].rearrange("p (k c) -> p k c",
                                                            c=NW))
                    yb = ybp.tile([128, ctj, PW], FP16, tag="yb")
                    for ct in range(ctj):
                        p4a = psum.tile([128, NW], F32, tag="p4a")
                        p4b = psum.tile([128, NW], F32, tag="p4b")
                        for k2 in range(MT):
                            nc.tensor.matmul(p4a[:],
                                             g_sl[j][:, k2, ct * 128:(ct + 1) * 128],
                                             w2a[:, k2, :],
                                             start=(k2 == 0), stop=(k2 == MT - 1))
                        for k2 in range(MT):
                            nc.tensor.matmul(p4b[:],
                                             g_sl[j][:, k2, ct * 128:(ct + 1) * 128],
                                             w2c[:, k2, :],
                                             start=(k2 == 0), stop=(k2 == MT - 1))
                        gbc = gat_sb[:, j, ct:ct + 1].to_broadcast([128, NW])
                        nc.vector.tensor_tensor(yb[:, ct, :NW], p4a[:], gbc,
                                                OP.mult)
                        nc.vector.tensor_tensor(yb[:, ct, NW:], p4b[:], gbc,
                                                OP.mult)
                    nc.gpsimd.dma_scatter_add(
                        y_p[p][:], yb[:], idxs_sb[:, j, :capj // 16],
                        capj, capj, PW)
                nc.gpsimd.collective_compute(
                    "ReduceScatter", OP.add,
                    replica_groups=[list(range(NCORES))],
                    ins=[y_p[p][0:T, :]],
                    outs=[rs_p[p][:]],
                )

            # ---------------- phase S: rest of shared intermediate ----------
            for sm in range(S1A, SIT):
                shared_int(sm)

            # ---------------- phase C: shared out + combine with rs ---------
            for n in range(H // NW):
                sh = []
                for hh in range(2):
                    s2 = wstream.tile([128, SIT // 2, NW], FP16, tag="w")
                    nc.scalar.dma_start(
                        s2[:], sw2b[n].rearrange("p (k c) -> p k c", c=NW)
                        [:, hh * (SIT // 2):(hh + 1) * (SIT // 2), :])
                    sh.append(s2)
                for ts in range(TS):
                    po = psum.tile([128, NW], F32, tag="p4a")
                    for k2 in range(SIT):
                        nc.tensor.matmul(po[:], gs[:, k2, ts * 128:(ts + 1) * 128],
                                         sh[k2 // (SIT // 2)][:, k2 % (SIT // 2), :],
                                         start=(k2 == 0), stop=(k2 == SIT - 1))
                    rst = small.tile([128, NW], FP16, tag="rst")
                    nc.sync.dma_start(
                        rst[:],
                        rs_p[n // 2][ts * 128:(ts + 1) * 128,
                                     (n % 2) * NW:(n % 2 + 1) * NW])
                    ott = small.tile([128, NW], F32, tag="ott")
                    nc.vector.tensor_tensor(ott[:], po[:], rst[:], OP.add)
                    nc.sync.dma_start(
                        out[ts * 128:(ts + 1) * 128, n * NW:(n + 1) * NW], ott[:])

    if compile_:
        nc.compile()
    else:
        nc.insert_library_loads()
    return nc


def host_prep(hidden_states, gate_weight, w1, w2, w3, sw1, sw2, sw3):
    B, S, H = hidden_states.shape
    T = B * S
    E, I = w1.shape[0], w1.shape[1]
    SI = sw1.shape[0]
    EPC = E // NCORES
    KT, MT, SIT = H // 128, I // 128, SI // 128
    N4 = max(H // 512, 1)
    NW = min(H, 512)
    TOUT = T // NCORES

    x = np.ascontiguousarray(hidden_states.reshape(T, H), dtype=np.float32)
    combine = _routing(x, gate_weight.astype(np.float32))
    tok_lists = [np.nonzero(combine[:, e])[0] for e in range(E)]
    counts = np.array([len(t) for t in tok_lists])

    # balance experts over cores: serpentine over count-sorted experts; slot s
    # on every core holds its s-th largest expert, so per-slot capacity is the
    # max over cores of that rank band.
    order = np.argsort(-counts, kind="stable")
    assign = np.zeros((NCORES, EPC), np.int64)
    for s in range(EPC):
        band = order[s * NCORES:(s + 1) * NCORES]
        if s % 2 == 1:
            band = band[::-1]
        for c in range(NCORES):
            assign[c, s] = band[c]
    CAPS = tuple(
        int(max(128, ((counts[assign[:, s]].max() + 127) // 128) * 128))
        for s in range(EPC))
    CAP0 = max(CAPS)
    CT0 = CAP0 // 128

    x16 = x.astype(np.float16)
    xT = x.T  # [H, T] view

    s1 = sw1.T.reshape(KT, 128, SIT, 128).transpose(2, 1, 0, 3)
    s3 = sw3.T.reshape(KT, 128, SIT, 128).transpose(2, 1, 0, 3)
    sw13 = np.ascontiguousarray(
        np.concatenate([s1, s3], axis=-1).reshape(SIT, 128, -1), dtype=np.float16)
    sw2b = np.ascontiguousarray(
        sw2.T.reshape(SIT, 128, N4, NW).transpose(2, 1, 0, 3).reshape(N4, 128, -1),
        dtype=np.float16)

    in_maps = []
    for c in range(NCORES):
        els = list(assign[c])
        idx_np = np.zeros((EPC, 128, CAP0 // 16), np.int16)
        idxs_np = np.zeros((EPC, 128, CAP0 // 16), np.int16)
        gat_np = np.zeros((EPC, 128, CT0), np.float32)
        for j, e in enumerate(els):
            cap = CAPS[j]
            toks = tok_lists[e]
            a = np.zeros(cap, np.int16)
            a[:len(toks)] = toks
            idx_np[j, :, :cap // 16] = np.tile(a.reshape(cap // 16, 16).T, (8, 1))
            b2 = np.full(cap, T, np.int16)  # pad rows land on dummy row T
            b2[:len(toks)] = toks
            idxs_np[j, :, :cap // 16] = np.tile(b2.reshape(cap // 16, 16).T, (8, 1))
            gv = np.zeros(cap, np.float32)
            gv[:len(toks)] = combine[toks, e]
            gat_np[j, :, :cap // 128] = gv.reshape(cap // 128, 128).T
        w13c = np.empty((EPC, MT, 128, KT * 256), np.float16)
        w2c = np.empty((EPC, N4, 128, MT * NW), np.float16)
        for j, e in enumerate(els):
            a1 = w1[e].T.reshape(KT, 128, MT, 128).transpose(2, 1, 0, 3)
            a3 = w3[e].T.reshape(KT, 128, MT, 128).transpose(2, 1, 0, 3)
            w13c[j] = np.concatenate([a1, a3], axis=-1).reshape(MT, 128, -1)
            w2c[j] = (w2[e].T.reshape(MT, 128, N4, NW)
                      .transpose(2, 1, 0, 3).reshape(N4, 128, -1))
        xTc = np.ascontiguousarray(
            xT[:, c * TOUT:(c + 1) * TOUT].reshape(KT, 128, TOUT)
            .transpose(1, 0, 2).reshape(128, -1), dtype=np.float16)
        in_maps.append({
            "x16": x16, "xTc": xTc,
            "w13": w13c, "w2b": w2c,
            "sw13": sw13, "sw2b": sw2b,
            "idx": idx_np, "idxs": idxs_np, "gat": gat_np,
        })
    cfg = dict(T=T, H=H, I=I, CAPS=CAPS, SI=SI)
    return in_maps, cfg


def kernel(**inputs):
    inputs = {k: np.asarray(v) for k, v in inputs.items()}
    hs = inputs["hidden_states"]
    B, S, H = hs.shape
    in_maps, cfg = host_prep(
        hs, inputs["gate_weight"], inputs["w1"], inputs["w2"], inputs["w3"],
        inputs["sw1"], inputs["sw2"], inputs["sw3"])
    nc = build_kernel(**cfg)
    res = run_bass_kernel_spmd(nc, in_maps, list(range(NCORES)))
    y = np.concatenate([res.results[c]["out"] for c in range(NCORES)], axis=0)
    return y.reshape(B, S, H).astype(np.float32)


if __name__ == "__main__":
    pass


# revision 11
# speedup vs baseline: 1.2251x; 1.0487x over previous
"""DeepSeekV2 MoE layer on 8 trn2 NeuronCores (expert-parallel).

Strategy (v5):
  - Host: gate softmax + group-limited top-k routing -> per-expert token index
    lists and combine weights (control data only; all heavy FLOPs on device).
    Experts are load-balanced across cores (serpentine over counts) and each
    core's 4 expert slots get per-slot capacities (max over cores, ceil 128).
  - Device (SPMD over 8 cores, 4 expert slots each):
      A: per slot: transposed fp16 dma_gather (double-buffered, spread over
         4 SWDGE queues) -> mm1/mm3 fp16 -> silu*mul -> g[slot] in SBUF;
         zero-fill of the y accumulators is interleaved here (sync/scalar);
      B: column-PAIR major down-proj: for each 1024-wide column pair: all 4
         slots' mm2 + gate-scale + one 2KB-elem dma_scatter_add per slot
         (negative-index padding skips pad tokens), then ReduceScatter(add)
         for the pair -> the 2 RS's overlap the shared-expert phase;
      S: shared-expert intermediate for own 512-token slice (after B so the
         RS chain hides under it; first S1A iters run up-front as warmup
         filler while the first gather lands);
      C: shared out matmuls + add RS result -> out.
  - Host: concatenate 512-row slices -> [B, S, H].
"""
import sys

import numpy as np

sys.path.insert(0, "/opt/trn_rl_repo")

import concourse.bass as bass
import concourse.mybir as mybir
import concourse.tile as tile
from concourse import bacc
from concourse.bass_utils import run_bass_kernel_spmd

F32 = mybir.dt.float32
FP16 = mybir.dt.float16
I16 = mybir.dt.int16
AF = mybir.ActivationFunctionType
OP = mybir.AluOpType

N_GROUP, TOPK_GROUP, TOP_K = 8, 3, 6
NCORES = 8
S1A = 5  # shared-intermediate iters run before phase A (warmup filler)


def _routing(x, gate_w):
    T, E = x.shape[0], gate_w.shape[0]
    logits = (x @ gate_w.T).astype(np.float64)
    e = np.exp(logits - logits.max(-1, keepdims=True))
    scores = e / e.sum(-1, keepdims=True)
    per_group = E // N_GROUP
    group_scores = scores.reshape(T, N_GROUP, per_group).max(-1)
    order = np.argsort(-group_scores, axis=-1, kind="stable")
    group_mask = np.zeros((T, N_GROUP), bool)
    np.put_along_axis(group_mask, order[:, :TOPK_GROUP], True, axis=1)
    tmp = np.where(np.repeat(group_mask, per_group, axis=1), scores, 0.0)
    order_e = np.argsort(-tmp, axis=-1, kind="stable")
    topk_idx = order_e[:, :TOP_K]
    topk_w = np.take_along_axis(tmp, topk_idx, axis=1)
    topk_w = topk_w / (topk_w.sum(-1, keepdims=True) + 1e-20)
    combine = np.zeros((T, E), np.float32)
    np.put_along_axis(combine, topk_idx, topk_w.astype(np.float32), axis=1)
    return combine


def _chunks(cap):
    out, rem = [], cap
    while rem:
        if rem <= 512:
            out.append(rem)
            rem = 0
        elif rem == 640:
            out.append(384)
            rem = 256
        else:
            out.append(512)
            rem -= 512
    return out


def build_kernel(T, H, I, CAPS, CAPS16, SI, act=AF.Silu, compile_=True):
    EPC = len(CAPS)
    KT = H // 128         # H contraction tiles
    MT = I // 128         # I tiles
    NP = max(H // 1024, 1)  # column pairs
    PW = min(H, 1024)       # pair width
    NW = 512
    SIT = SI // 128       # shared-intermediate tiles
    TOUT = T // NCORES    # own token slice
    TS = TOUT // 128
    CAP0 = max(CAPS)
    CT0 = CAP0 // 128
    CHUNKS = [_chunks(c) for c in CAPS]
    ZBLK = (T + 128) // 128  # zero blocks per pair tensor

    nc = bacc.Bacc("TRN2")
    x16 = nc.dram_tensor("x16", [T, H], FP16, kind="ExternalInput")
    xTc = nc.dram_tensor("xTc", [128, KT * TOUT], FP16, kind="ExternalInput")
    w13 = nc.dram_tensor("w13", [EPC, MT, 128, KT * 256], FP16, kind="ExternalInput")
    w2b = nc.dram_tensor("w2b", [EPC, H // NW, 128, MT * NW], FP16,
                         kind="ExternalInput")
    sw13 = nc.dram_tensor("sw13", [SIT, 128, KT * 256], FP16, kind="ExternalInput")
    sw2b = nc.dram_tensor("sw2b", [H // NW, 128, SIT * NW], FP16,
                          kind="ExternalInput")
    idx = nc.dram_tensor("idx", [EPC, 128, CAP0 // 16], I16, kind="ExternalInput")
    idxs = nc.dram_tensor("idxs", [EPC, 128, CAP0 // 16], I16, kind="ExternalInput")
    gat = nc.dram_tensor("gat", [EPC, 128, CT0], F32, kind="ExternalInput")
    out = nc.dram_tensor("out", [TOUT, H], FP16, kind="ExternalOutput")

    y_p = [nc.dram_tensor(f"y_pair{p}", [T + 128, PW], FP16) for p in range(NP)]
    rs_p = [nc.dram_tensor(f"rs_pair{p}", [TOUT, PW], FP16) for p in range(NP)]

    with tile.TileContext(nc) as tc:
        with (
            tc.tile_pool(name="const", bufs=1) as const,
            tc.tile_pool(name="persist", bufs=1) as persist,
            tc.tile_pool(name="xgtp", bufs=2) as xgtp,
            tc.tile_pool(name="wstream", bufs=4) as wstream,
            tc.tile_pool(name="ybp", bufs=2) as ybp,
            tc.tile_pool(name="small", bufs=2) as small,
            tc.tile_pool(name="psum", bufs=2, space="PSUM") as psum,
        ):
            idx_sb = const.tile([128, EPC, CAP0 // 16], I16)
            nc.scalar.dma_start(idx_sb[:], idx.rearrange("e p c -> p e c"))
            idxs_sb = const.tile([128, EPC, CAP0 // 16], I16)
            nc.scalar.dma_start(idxs_sb[:], idxs.rearrange("e p c -> p e c"))
            gat_sb = const.tile([128, EPC, CT0], F32)
            nc.scalar.dma_start(gat_sb[:], gat.rearrange("e p c -> p e c"))
            # shared-expert input (own tokens, H-tiled on partitions)
            xtc_sb = persist.tile([128, KT, TOUT], FP16)
            nc.scalar.dma_start(xtc_sb[:], xTc[:])
            gs = persist.tile([128, SIT, TOUT], FP16)
            g_sl = [persist.tile([128, MT, CAPS16[j]], FP16, tag=f"g{j}",
                                 name=f"g{j}")
                    for j in range(EPC)]
            ztile = const.tile([128, NW], FP16)
            nc.vector.memset(ztile[:], 0.0)

            def shared_int(sm):
                s13 = wstream.tile([128, KT, 256], FP16, tag="w")
                nc.sync.dma_start(
                    s13[:], sw13[sm].rearrange("p (k c) -> p k c", c=256))
                p1 = psum.tile([128, 512], F32, tag="p1")
                p3 = psum.tile([128, 512], F32, tag="p3")
                for k in range(KT):
                    nc.tensor.matmul(p1[:, :TOUT], s13[:, k, :128], xtc_sb[:, k, :],
                                     start=(k == 0), stop=(k == KT - 1))
                for k in range(KT):
                    nc.tensor.matmul(p3[:, :TOUT], s13[:, k, 128:], xtc_sb[:, k, :],
                                     start=(k == 0), stop=(k == KT - 1))
                nc.scalar.activation(gs[:, sm, :], p1[:, :TOUT], act)
                nc.vector.tensor_tensor(gs[:, sm, :], gs[:, sm, :], p3[:, :TOUT],
                                        OP.mult)

            # warmup filler while the first gathers land
            for sm in range(S1A):
                shared_int(sm)

            # ---------------- phase A: gathers + up-proj -> g ---------------
            # (zero-fill of y pairs is interleaved per expert: sync/scalar)
            for j in range(EPC):
                xgt_c = []
                c0 = 0
                for ci, cw in enumerate(CHUNKS[j]):
                    xgt = xgtp.tile([128, KT, cw], FP16, tag=f"xg{ci}",
                                    name=f"xg{ci}")
                    nc.gpsimd.dma_gather(
                        xgt[:], x16[:],
                        idx_sb[:, j, c0 // 16:(c0 + cw) // 16],
                        cw, cw, H, transpose=True)
                    xgt_c.append(xgt)
                    c0 += cw
                for m in range(MT):
                    w13t = wstream.tile([128, KT, 256], FP16, tag="w")
                    nc.sync.dma_start(
                        w13t[:], w13[j, m].rearrange("p (k c) -> p k c", c=256))
                    c0 = 0
                    for ci, cw in enumerate(CHUNKS[j]):
                        # compute width trimmed to the 16-granular capacity;
                        # the g tail [cwc, cw) stays garbage -> zero gate ->
                        # scatters to the dummy row.
                        cwc = min(cw, CAPS16[j] - c0)
                        p1 = psum.tile([128, 512], F32, tag="p1")
                        p3 = psum.tile([128, 512], F32, tag="p3")
                        for k in range(KT):
                            nc.tensor.matmul(p1[:, :cwc], w13t[:, k, :128],
                                             xgt_c[ci][:, k, :cwc],
                                             start=(k == 0), stop=(k == KT - 1))
                        for k in range(KT):
                            nc.tensor.matmul(p3[:, :cwc], w13t[:, k, 128:],
                                             xgt_c[ci][:, k, :cwc],
                                             start=(k == 0), stop=(k == KT - 1))
                        nc.scalar.activation(g_sl[j][:, m, c0:c0 + cwc],
                                             p1[:, :cwc], act)
                        nc.vector.tensor_tensor(g_sl[j][:, m, c0:c0 + cwc],
                                                g_sl[j][:, m, c0:c0 + cwc],
                                                p3[:, :cwc], OP.mult)
                        c0 += cw
                # zero-fill: expert j covers half the blocks of pair j//2
                pz = j // (EPC // NP) if NP > 1 else 0
                half = j % (EPC // NP)
                blocks = range(half * ((ZBLK + 1) // 2),
                               min(ZBLK, (half + 1) * ((ZBLK + 1) // 2)))
                for bi, b in enumerate(blocks):
                    eng = nc.sync if bi % 2 == 0 else nc.scalar
                    eng.dma_start(y_p[pz][b * 128:(b + 1) * 128, :NW], ztile[:])
                    eng2 = nc.scalar if bi % 2 == 0 else nc.sync
                    eng2.dma_start(y_p[pz][b * 128:(b + 1) * 128, NW:], ztile[:])

            # ---------------- phase B: pair-major down-proj + per-pair RS ---
            for p in range(NP):
                for j in range(EPC):
                    capj = CAPS[j]
                    ctj = capj // 128
                    w2a = wstream.tile([128, MT, NW], FP16, tag="w")
                    nc.scalar.dma_start(
                        w2a[:], w2b[j, 2 * p].rearrange("p (k c) -> p k c", c=NW))
                    w2c = wstream.tile([128, MT, NW], FP16, tag="w")
                    nc.scalar.dma_start(
                        w2c[:], w2b[j, 2 * p + 1].rearrange("p (k c) -> p k c",
                                                            c=NW))
                    yb = ybp.tile([128, ctj, PW], FP16, tag="yb")
                    for ct in range(ctj):
                        ctw = min(128, CAPS16[j] - ct * 128)
                        p4a = psum.tile([128, NW], F32, tag="p4a")
                        p4b = psum.tile([128, NW], F32, tag="p4b")
                        for k2 in range(MT):
                            nc.tensor.matmul(p4a[:ctw],
                                             g_sl[j][:, k2,
                                                     ct * 128:ct * 128 + ctw],
                                             w2a[:, k2, :],
                                             start=(k2 == 0), stop=(k2 == MT - 1))
                        for k2 in range(MT):
                            nc.tensor.matmul(p4b[:ctw],
                                             g_sl[j][:, k2,
                                                     ct * 128:ct * 128 + ctw],
                                             w2c[:, k2, :],
                                             start=(k2 == 0), stop=(k2 == MT - 1))
                        gbc = gat_sb[:, j, ct:ct + 1].to_broadcast([128, NW])
                        nc.vector.tensor_tensor(yb[:, ct, :NW], p4a[:], gbc,
                                                OP.mult)
                        nc.vector.tensor_tensor(yb[:, ct, NW:], p4b[:], gbc,
                                                OP.mult)
                    nc.gpsimd.dma_scatter_add(
                        y_p[p][:], yb[:], idxs_sb[:, j, :capj // 16],
                        capj, capj, PW)
                nc.gpsimd.collective_compute(
                    "ReduceScatter", OP.add,
                    replica_groups=[list(range(NCORES))],
                    ins=[y_p[p][0:T, :]],
                    outs=[rs_p[p][:]],
                )

            # ---------------- phase S: rest of shared intermediate ----------
            for sm in range(S1A, SIT):
                shared_int(sm)

            # ---------------- phase C: shared out + combine with rs ---------
            for n in range(H // NW):
                sh = []
                for hh in range(2):
                    s2 = wstream.tile([128, SIT // 2, NW], FP16, tag="w")
                    nc.scalar.dma_start(
                        s2[:], sw2b[n].rearrange("p (k c) -> p k c", c=NW)
                        [:, hh * (SIT // 2):(hh + 1) * (SIT // 2), :])
                    sh.append(s2)
                for ts in range(TS):
                    po = psum.tile([128, NW], F32, tag="p4a")
                    for k2 in range(SIT):
                        nc.tensor.matmul(po[:], gs[:, k2, ts * 128:(ts + 1) * 128],
                                         sh[k2 // (SIT // 2)][:, k2 % (SIT // 2), :],
                                         start=(k2 == 0), stop=(k2 == SIT - 1))
                    rst = small.tile([128, NW], FP16, tag="rst")
                    nc.sync.dma_start(
                        rst[:],
                        rs_p[n // 2][ts * 128:(ts + 1) * 128,
                                     (n % 2) * NW:(n % 2 + 1) * NW])
                    ott = small.tile([128, NW], FP16, tag="ott")
                    nc.vector.tensor_tensor(ott[:], po[:], rst[:], OP.add)
                    nc.sync.dma_start(
                        out[ts * 128:(ts + 1) * 128, n * NW:(n + 1) * NW], ott[:])

    if compile_:
        nc.compile()
    else:
        nc.insert_library_loads()
    return nc


def host_prep(hidden_states, gate_weight, w1, w2, w3, sw1, sw2, sw3):
    B, S, H = hidden_states.shape
    T = B * S
    E, I = w1.shape[0], w1.shape[1]
    SI = sw1.shape[0]
    EPC = E // NCORES
    KT, MT, SIT = H // 128, I // 128, SI // 128
    N4 = max(H // 512, 1)
    NW = min(H, 512)
    TOUT = T // NCORES

    x = np.ascontiguousarray(hidden_states.reshape(T, H), dtype=np.float32)
    combine = _routing(x, gate_weight.astype(np.float32))
    tok_lists = [np.nonzero(combine[:, e])[0] for e in range(E)]
    counts = np.array([len(t) for t in tok_lists])

    # balance experts over cores: serpentine over count-sorted experts; slot s
    # on every core holds its s-th largest expert, so per-slot capacity is the
    # max over cores of that rank band.
    order = np.argsort(-counts, kind="stable")
    assign = np.zeros((NCORES, EPC), np.int64)
    for s in range(EPC):
        band = order[s * NCORES:(s + 1) * NCORES]
        if s % 2 == 1:
            band = band[::-1]
        for c in range(NCORES):
            assign[c, s] = band[c]
    CAPS = tuple(
        int(max(128, ((counts[assign[:, s]].max() + 127) // 128) * 128))
        for s in range(EPC))
    CAPS16 = tuple(
        int(max(128, ((counts[assign[:, s]].max() + 15) // 16) * 16))
        for s in range(EPC))
    CAP0 = max(CAPS)
    CT0 = CAP0 // 128

    x16 = x.astype(np.float16)
    xT = x.T  # [H, T] view

    s1 = sw1.T.reshape(KT, 128, SIT, 128).transpose(2, 1, 0, 3)
    s3 = sw3.T.reshape(KT, 128, SIT, 128).transpose(2, 1, 0, 3)
    sw13 = np.ascontiguousarray(
        np.concatenate([s1, s3], axis=-1).reshape(SIT, 128, -1), dtype=np.float16)
    sw2b = np.ascontiguousarray(
        sw2.T.reshape(SIT, 128, N4, NW).transpose(2, 1, 0, 3).reshape(N4, 128, -1),
        dtype=np.float16)

    in_maps = []
    for c in range(NCORES):
        els = list(assign[c])
        idx_np = np.zeros((EPC, 128, CAP0 // 16), np.int16)
        idxs_np = np.zeros((EPC, 128, CAP0 // 16), np.int16)
        gat_np = np.zeros((EPC, 128, CT0), np.float32)
        for j, e in enumerate(els):
            cap = CAPS[j]
            toks = tok_lists[e]
            a = np.zeros(cap, np.int16)
            a[:len(toks)] = toks
            idx_np[j, :, :cap // 16] = np.tile(a.reshape(cap // 16, 16).T, (8, 1))
            b2 = np.full(cap, T, np.int16)  # pad rows land on dummy row T
            b2[:len(toks)] = toks
            idxs_np[j, :, :cap // 16] = np.tile(b2.reshape(cap // 16, 16).T, (8, 1))
            gv = np.zeros(cap, np.float32)
            gv[:len(toks)] = combine[toks, e]
            gat_np[j, :, :cap // 128] = gv.reshape(cap // 128, 128).T
        w13c = np.empty((EPC, MT, 128, KT * 256), np.float16)
        w2c = np.empty((EPC, N4, 128, MT * NW), np.float16)
        for j, e in enumerate(els):
            a1 = w1[e].T.reshape(KT, 128, MT, 128).transpose(2, 1, 0, 3)
            a3 = w3[e].T.reshape(KT, 128, MT, 128).transpose(2, 1, 0, 3)
            w13c[j] = np.concatenate([a1, a3], axis=-1).reshape(MT, 128, -1)
            w2c[j] = (w2[e].T.reshape(MT, 128, N4, NW)
                      .transpose(2, 1, 0, 3).reshape(N4, 128, -1))
        xTc = np.ascontiguousarray(
            xT[:, c * TOUT:(c + 1) * TOUT].reshape(KT, 128, TOUT)
            .transpose(1, 0, 2).reshape(128, -1), dtype=np.float16)
        in_maps.append({
            "x16": x16, "xTc": xTc,
            "w13": w13c, "w2b": w2c,
            "sw13": sw13, "sw2b": sw2b,
            "idx": idx_np, "idxs": idxs_np, "gat": gat_np,
        })
    cfg = dict(T=T, H=H, I=I, CAPS=CAPS, CAPS16=CAPS16, SI=SI)
    return in_maps, cfg


def kernel(**inputs):
    inputs = {k: np.asarray(v) for k, v in inputs.items()}
    hs = inputs["hidden_states"]
    B, S, H = hs.shape
    in_maps, cfg = host_prep(
        hs, inputs["gate_weight"], inputs["w1"], inputs["w2"], inputs["w3"],
        inputs["sw1"], inputs["sw2"], inputs["sw3"])
    nc = build_kernel(**cfg)
    res = run_bass_kernel_spmd(nc, in_maps, list(range(NCORES)))
    y = np.concatenate([res.results[c]["out"] for c in range(NCORES)], axis=0)
    return y.reshape(B, S, H).astype(np.float32)


if __name__ == "__main__":
    pass
